# revision 1
# baseline (speedup 1.0000x reference)
"""BitNet transformer block kernel for 8 Trainium2 NeuronCores.

Sharding: data-parallel. Core c handles batch c//4, token chunk c%4 (512
query tokens). Each core computes K/V for its batch's full 2048-token
sequence (redundant KV compute instead of collectives). Host rotates the
token axis per core so every core's own tokens sit at chunk 0 -> all 8
cores run one identical SPMD program.

Layout: activations feature-major (x^T tiles [128 feat, T tok]) so matmul
contraction (features) lies on partitions. V is token-major for the AV
matmul. Weights host-prepped: quantized, scales/LN-gamma folded in; biases
folded/propagated. Q/K/logit path in float32r (TF32-like, ~1e-4 rel),
everything else bf16 with fp32 accumulation.
"""
import sys

sys.path.insert(0, "/opt/trn_rl_repo")

import numpy as np
import ml_dtypes
from contextlib import ExitStack

import concourse.bass as bass
import concourse.bacc as bacc
import concourse.tile as tile
from concourse import mybir
from concourse.bass_utils import run_bass_kernel_spmd
from concourse.masks import make_identity

F32 = mybir.dt.float32
F32R = mybir.dt.float32r
BF16 = mybir.dt.bfloat16
AF = mybir.ActivationFunctionType
AX = mybir.AxisListType

DIM = 1024
HEADS = 16
DH = 64
FF = 4096
EPS = 1e-5
T = 2048        # tokens per batch (per-core KV scope)
NQ = 512        # own query tokens per core
KD = DIM // 128   # 8 feature tiles
CHUNK = 512
NCHUNK = T // CHUNK  # 4
N_CORES = 8

_cache = {}


def _quantize(w):
    w = w.astype(np.float32)
    return np.round(np.clip(w, -2.0, 2.0) * np.float32(0.75) + np.float32(0.5)) - np.float32(0.5)


def _make_e4():
    e4 = np.zeros((4, 256), np.float32)
    for qt in range(4):
        e4[qt, qt * 64:(qt + 1) * 64] = 1.0
    return e4


def _prep_weights(i):
    """Host-side: quantize, fold scales/LN-params, transpose to [in, out]."""
    q = {k: _quantize(i[k]) for k in ("wq", "wk", "wv", "wo", "w1", "w2")}
    Wq = q["wq"] * i["sq"][:, None]
    Wk = q["wk"] * i["sk"][:, None]
    Wv = q["wv"] * i["sv"][:, None]
    Wo = q["wo"] * i["so"][:, None]
    W1 = q["w1"] * i["s1"][:, None]
    W2 = q["w2"] * i["s2"][:, None]
    g1, b1n = i["ln1_g"], i["ln1_b"]
    g2, b2n = i["ln2_g"], i["ln2_b"]
    s8 = np.float32(DH ** -0.5)
    out = {}
    out["wqT"] = np.ascontiguousarray((Wq * g1[None, :] * s8).T.astype(np.float32))
    out["bq"] = ((Wq @ b1n + i["bq"]) * s8).astype(np.float32)
    out["wkT"] = np.ascontiguousarray((Wk * g1[None, :]).T.astype(np.float32))
    out["bk"] = (Wk @ b1n + i["bk"]).astype(np.float32)
    out["wvT"] = np.ascontiguousarray((Wv * g1[None, :]).T.astype(ml_dtypes.bfloat16))
    bv = Wv @ b1n + i["bv"]
    out["woT"] = np.ascontiguousarray(Wo.T.astype(ml_dtypes.bfloat16))
    out["bo"] = (Wo @ bv + i["bo"]).astype(np.float32)
    out["w1T"] = np.ascontiguousarray((W1 * g2[None, :]).T.astype(ml_dtypes.bfloat16))
    out["b1"] = (W1 @ b2n + i["b1"]).astype(np.float32)
    out["w2T"] = np.ascontiguousarray(W2.T.astype(ml_dtypes.bfloat16))
    out["b2"] = i["b2"].astype(np.float32)
    out["e4"] = _make_e4()
    return out


def _bcast_ap(t):
    """(unused) Partition-broadcast read AP of a [1, N] sbuf tile."""
    return bass.AP(tensor=t.tensor, offset=t.offset,
                   ap=[[0, 128]] + [list(a) for a in t.ap[1:]])


def _ln_chunk(nc, sb, ps_bc, scratch, ps_stat, xh_pool, xt, ones, ones1, width,
              out_dt=F32R):
    """LayerNorm transform of one feature-major chunk [128, KD, width].
    Returns xh = (x - mu) * rstd."""
    ssum = ps_stat.tile([1, width], F32, name="ssum")
    ssq = ps_stat.tile([1, width], F32, name="ssq")
    for k in range(KD):
        sq = scratch.tile([128, width], F32R, name="scr", tag="scr")
        nc.scalar.activation(sq[:], xt[:, k], AF.Square)
        nc.tensor.matmul(ssum[:], lhsT=ones[:], rhs=xt[:, k],
                         start=(k == 0), stop=(k == KD - 1))
        nc.tensor.matmul(ssq[:], lhsT=ones[:], rhs=sq[:],
                         start=(k == 0), stop=(k == KD - 1))
    mu = sb.tile([1, width], F32, name="mu")
    nc.vector.tensor_scalar_mul(mu[:], ssum[:], 1.0 / DIM)
    var = sb.tile([1, width], F32, name="var")
    # var = ssq/DIM - mu^2  via (ssq*(1/DIM) - mu*mu)
    musq = sb.tile([1, width], F32, name="musq")
    nc.vector.tensor_mul(musq[:], mu[:], mu[:])
    nc.vector.tensor_scalar(var[:], ssq[:], 1.0 / DIM, None,
                            mybir.AluOpType.mult)
    nc.vector.tensor_sub(var[:], var[:], musq[:])
    nc.vector.tensor_scalar_add(var[:], var[:], float(EPS))
    sd = sb.tile([1, width], F32, name="sd")
    nc.scalar.activation(sd[:], var[:], AF.Sqrt)
    r = sb.tile([1, width], F32, name="r")
    nc.vector.reciprocal(r[:], sd[:])
    mu_b = ps_bc.tile([128, width], F32, name="mu_b")
    r_b = ps_bc.tile([128, width], F32, name="r_b")
    nc.tensor.matmul(mu_b[:], lhsT=ones1[:], rhs=mu[:], start=True, stop=True)
    nc.tensor.matmul(r_b[:], lhsT=ones1[:], rhs=r[:], start=True, stop=True)
    xh = xh_pool.tile([128, KD, width], out_dt, name="xh")
    for k in range(KD):
        xc = scratch.tile([128, width], F32, name="scr2", tag="scr")
        nc.vector.tensor_sub(xc[:], xt[:, k], mu_b[:])
        nc.vector.tensor_mul(xh[:, k], xc[:], r_b[:])
    return xh


def _wslice(d, name, m, mm=128):
    """[DIM_in, n_out] weight dram -> lhsT tile view [128, KD_in, mm] for
    out-block m."""
    return d[name].rearrange("(k p) (mb mm) -> p k mb mm", p=128, mm=mm)[:, :, m]


def _body(nc, tc, d):
    ctx = ExitStack()
    with ctx:
        const = ctx.enter_context(tc.tile_pool(name="const", bufs=1))
        ones_blk = const.tile([128, 128], F32, name="ones_blk")
        nc.vector.memset(ones_blk[:], 1.0)
        ones1 = ones_blk[0:1, :]
        ones = const.tile([128, 1], F32R, name="ones")
        nc.vector.tensor_copy(ones[:], ones_blk[:, 0:1])
        ident = const.tile([128, 128], BF16, name="ident")
        make_identity(nc, ident)

        bias = {}
        for nm, n in [("bq", DIM), ("bk", DIM), ("bo", DIM), ("b1", FF), ("b2", DIM)]:
            t = const.tile([128, n // 128], F32, name=f"sb_{nm}")
            nc.sync.dma_start(out=t[:], in_=d[nm].rearrange("(m p) -> p m", p=128))
            bias[nm] = t

        # long-lived activations
        KT_sb = const.tile([128, KD, T], F32R, name="KT_sb")        # 8MB
        V_sb = const.tile([128, T // 128, DIM], BF16, name="V_sb")  # 4MB
        QT_sb = const.tile([128, KD, NQ], F32R, name="QT_sb")       # 2MB
        ATTN_mbs = [const.tile([128, NQ], BF16, name=f"ATTN_{i}") for i in range(KD)]

        xT_t = d["xT"].rearrange("(k p) t -> p k t", p=128)

        # ---- Phase A: LN1 + K/V proj per chunk; Q proj on chunk 0 ----
        with ExitStack() as actx:
            sb_ln = actx.enter_context(tc.tile_pool(name="sb_ln", bufs=1))
            ps_bc = actx.enter_context(tc.tile_pool(name="ps_bc", bufs=1, space="PSUM"))
            scratch = actx.enter_context(tc.tile_pool(name="scratch", bufs=2))
            sb_xt = actx.enter_context(tc.tile_pool(name="sb_xt", bufs=1))
            sb_xh = actx.enter_context(tc.tile_pool(name="sb_xh", bufs=2))
            sb_xhbf = actx.enter_context(tc.tile_pool(name="sb_xhbf", bufs=1))
            wstr = actx.enter_context(tc.tile_pool(name="wstr", bufs=2))
            ps_stat = actx.enter_context(tc.tile_pool(name="ps_stat", bufs=1, space="PSUM"))
            ps_mm = actx.enter_context(tc.tile_pool(name="ps_mm", bufs=4, space="PSUM"))

            for c in range(NCHUNK):
                xt = sb_xt.tile([128, KD, CHUNK], F32R, name="xt")
                nc.sync.dma_start(out=xt[:], in_=xT_t[:, :, c * CHUNK:(c + 1) * CHUNK])
                xh = _ln_chunk(nc, sb_ln, ps_bc, scratch, ps_stat, sb_xh, xt, ones, ones1, CHUNK)
                xh_bf = sb_xhbf.tile([128, KD, CHUNK], BF16, name="xh_bf")
                nc.vector.tensor_copy(xh_bf[:], xh[:])

                for m in range(KD):
                    wk = wstr.tile([128, KD, 128], F32R, name="wk", tag="wk")
                    nc.sync.dma_start(out=wk[:], in_=_wslice(d, "wkT", m))
                    kp = ps_mm.tile([128, CHUNK], F32, name="kp", tag="mm")
                    for k in range(KD):
                        nc.tensor.matmul(kp[:], lhsT=wk[:, k], rhs=xh[:, k],
                                         start=(k == 0), stop=(k == KD - 1))
                    nc.scalar.activation(KT_sb[:, m, c * CHUNK:(c + 1) * CHUNK], kp[:],
                                         AF.Identity, bias=bias["bk"][:, m:m + 1])
                wvT_v = d["wvT"].rearrange("(kh k p) (nb nn) -> p kh k nb nn",
                                           p=128, k=4, nn=CHUNK)
                for nb in range(2):
                    wvs = []
                    for kh in range(2):
                        wv = wstr.tile([128, 4, CHUNK], BF16, name="wv", tag="wv")
                        nc.sync.dma_start(out=wv[:], in_=wvT_v[:, kh, :, nb])
                        wvs.append(wv)
                    for t_sub in range(CHUNK // 128):
                        blk = c * 4 + t_sub
                        vp = ps_mm.tile([128, CHUNK], F32, name="vp", tag="mm")
                        for k in range(KD):
                            nc.tensor.matmul(
                                vp[:], lhsT=xh_bf[:, k, t_sub * 128:(t_sub + 1) * 128],
                                rhs=wvs[k // 4][:, k % 4], start=(k == 0), stop=(k == KD - 1))
                        nc.vector.tensor_copy(
                            V_sb[:, blk, nb * CHUNK:(nb + 1) * CHUNK], vp[:])
                if c == 0:
                    for m in range(KD):
                        wq = wstr.tile([128, KD, 128], F32R, name="wq", tag="wk")
                        nc.sync.dma_start(out=wq[:], in_=_wslice(d, "wqT", m))
                        qp = ps_mm.tile([128, CHUNK], F32, name="qp", tag="mm")
                        for k in range(KD):
                            nc.tensor.matmul(qp[:], lhsT=wq[:, k], rhs=xh[:, k],
                                             start=(k == 0), stop=(k == KD - 1))
                        nc.scalar.activation(QT_sb[:, m], qp[:],
                                             AF.Identity, bias=bias["bq"][:, m:m + 1])

        # ---- Phase B: attention ----
        with ExitStack() as bctx:
            sb_A = bctx.enter_context(tc.tile_pool(name="sb_A", bufs=6))
            sb_AT = bctx.enter_context(tc.tile_pool(name="sb_AT", bufs=2))
            sb_sm = bctx.enter_context(tc.tile_pool(name="sb_sm", bufs=4))
            sb_rb = bctx.enter_context(tc.tile_pool(name="sb_rb", bufs=2))
            ps_S = bctx.enter_context(tc.tile_pool(name="ps_S", bufs=2, space="PSUM"))
            sb_S = bctx.enter_context(tc.tile_pool(name="sb_S", bufs=3))
            ps_av = bctx.enter_context(tc.tile_pool(name="ps_av", bufs=1, space="PSUM"))
            ps_trb = bctx.enter_context(tc.tile_pool(name="ps_trb", bufs=1, space="PSUM"))
            constB = bctx.enter_context(tc.tile_pool(name="constB", bufs=1))
            ident_f32 = constB.tile([128, 128], F32, name="ident_f32")
            make_identity(nc, ident_f32)
            e4_sb = constB.tile([4, 256], F32, name="e4_sb")
            nc.sync.dma_start(out=e4_sb[:], in_=d["e4"][:])
            ps_tp = bctx.enter_context(tc.tile_pool(name="ps_tp", bufs=2, space="PSUM"))


            for h in range(HEADS):
                mb, r0 = h // 2, (h % 2) * 64
                A_qts = []
                for qt in range(NQ // 128):
                    A = sb_A.tile([128, T], BF16, name="A")
                    A_qts.append(A)
                    q_sl = QT_sb[r0:r0 + 64, mb, qt * 128:(qt + 1) * 128]
                    scp = sb_S.tile([128, 2, 2 * CHUNK], F32, name="scp")
                    for jj in range(2):
                        S = ps_S.tile([128, 2, CHUNK], F32, name="S")
                        for j in range(2):
                            k_off = (jj * 2 + j) * CHUNK
                            nc.tensor.matmul(
                                S[:, j], lhsT=q_sl,
                                rhs=KT_sb[r0:r0 + 64, mb, k_off:k_off + CHUNK],
                                start=True, stop=True)
                        nc.scalar.copy(scp[:, jj], S.rearrange("p a b -> p (a b)"))
                    scpf = scp.rearrange("p a b -> p (a b)")
                    negm = sb_sm.tile([128, 1], F32, name="negm")
                    nc.vector.reduce_max(negm[:], scpf[:], axis=AX.X)
                    nc.vector.tensor_scalar_mul(negm[:], negm[:], -1.0)
                    den = sb_sm.tile([128, 1], F32, name="den")
                    nc.scalar.activation(A[:], scpf[:], AF.Exp,
                                         bias=negm[:], accum_out=den[:])
                    rden = sb_sm.tile([128, 1], F32, name="rden")
                    nc.vector.reciprocal(rden[:], den[:])
                    nc.vector.tensor_scalar_mul(A[:], A[:], rden[:])

                AT = sb_AT.tile([128, T // 128, NQ], BF16, name="AT")
                for qt in range(NQ // 128):
                    for half in range(2):
                        tp = ps_tp.tile([128, 8, 128], BF16, name="tp")
                        for kb8 in range(8):
                            kb = half * 8 + kb8
                            nc.tensor.transpose(
                                tp[:, kb8], A_qts[qt][:, kb * 128:(kb + 1) * 128], ident[:])
                        nc.vector.tensor_copy(
                            AT[:, half * 8:(half + 1) * 8, qt * 128:(qt + 1) * 128],
                            tp[:])
                av = ps_av.tile([64, NQ], F32, name="av")
                for kb in range(T // 128):
                    nc.tensor.matmul(av[:], lhsT=V_sb[:, kb, h * DH:(h + 1) * DH],
                                     rhs=AT[:, kb],
                                     start=(kb == 0), stop=(kb == T // 128 - 1))
                nc.scalar.copy(ATTN_mbs[mb][r0:r0 + 64, :], av[:])

        # ---- Phase C: O proj + residual + LN2 + FF ----
        with ExitStack() as cctx:
            sb_ln2 = cctx.enter_context(tc.tile_pool(name="sb_ln2", bufs=1))
            ps_bc2 = cctx.enter_context(tc.tile_pool(name="ps_bc2", bufs=1, space="PSUM"))
            scr2 = cctx.enter_context(tc.tile_pool(name="scr2", bufs=2))
            sb_u = cctx.enter_context(tc.tile_pool(name="sb_u", bufs=1))
            wstr2 = cctx.enter_context(tc.tile_pool(name="wstr2", bufs=2))
            ps_stat2 = cctx.enter_context(tc.tile_pool(name="ps_stat2", bufs=1, space="PSUM"))
            ps_mm2 = cctx.enter_context(tc.tile_pool(name="ps_mm2", bufs=4, space="PSUM"))

            sb_xr = cctx.enter_context(tc.tile_pool(name="sb_xr", bufs=2))
            u_sb = sb_u.tile([128, KD, NQ], F32R, name="u_sb")
            for m in range(KD):
                wo = wstr2.tile([128, KD, 128], BF16, name="wo", tag="wsm")
                nc.sync.dma_start(out=wo[:], in_=_wslice(d, "woT", m))
                op = ps_mm2.tile([128, NQ], F32, name="op", tag="mm")
                for k in range(KD):
                    nc.tensor.matmul(op[:], lhsT=wo[:, k], rhs=ATTN_mbs[k][:],
                                     start=(k == 0), stop=(k == KD - 1))
                xr = sb_xr.tile([128, NQ], F32R, name="xr", tag="xr")
                nc.sync.dma_start(out=xr[:], in_=xT_t[:, m, 0:NQ])
                upre = scr2.tile([128, NQ], F32, name="upre", tag="scr")
                nc.vector.tensor_add(upre[:], op[:], xr[:])
                nc.scalar.activation(u_sb[:, m], upre[:], AF.Identity,
                                     bias=bias["bo"][:, m:m + 1])
            uh_bf = _ln_chunk(nc, sb_ln2, ps_bc2, scr2, ps_stat2, sb_u, u_sb, ones,
                              ones1, NQ, out_dt=BF16)
            H_sb = sb_u.tile([128, FF // 128, NQ], BF16, name="H_sb")
            for m in range(FF // 128):
                w1 = wstr2.tile([128, KD, 128], BF16, name="w1", tag="wsm")
                nc.sync.dma_start(out=w1[:], in_=_wslice(d, "w1T", m))
                h1 = ps_mm2.tile([128, NQ], F32, name="h1", tag="mm")
                for k in range(KD):
                    nc.tensor.matmul(h1[:], lhsT=w1[:, k], rhs=uh_bf[:, k],
                                     start=(k == 0), stop=(k == KD - 1))
                nc.scalar.activation(H_sb[:, m], h1[:], AF.Gelu,
                                     bias=bias["b1"][:, m:m + 1])
            w2T_v = d["w2T"].rearrange("(kh k p) (mb mm) -> p kh k mb mm",
                                       p=128, k=8, mm=128)
            for m in range(KD):
                f2 = ps_mm2.tile([128, NQ], F32, name="f2", tag="mm")
                for kh in range(4):
                    w2 = wstr2.tile([128, 8, 128], BF16, name="w2", tag="w2")
                    nc.sync.dma_start(out=w2[:], in_=w2T_v[:, kh, :, m])
                    for k in range(8):
                        nc.tensor.matmul(f2[:], lhsT=w2[:, k], rhs=H_sb[:, kh * 8 + k],
                                         start=(kh == 0 and k == 0),
                                         stop=(kh == 3 and k == 7))
                opre = scr2.tile([128, NQ], F32, name="opre", tag="scr")
                nc.vector.tensor_add(opre[:], f2[:], u_sb[:, m])
                oout = scr2.tile([128, NQ], F32, name="oout", tag="scr")
                nc.scalar.activation(oout[:], opre[:], AF.Identity,
                                     bias=bias["b2"][:, m:m + 1])
                nc.sync.dma_start(out=d["yT"][m * 128:(m + 1) * 128, :], in_=oout[:])


def _build():
    nc = bacc.Bacc("TRN2", target_bir_lowering=False, debug=False,
                   num_devices=N_CORES)
    d = {}
    d["xT"] = nc.dram_tensor("xT", [DIM, T], F32R, kind="ExternalInput").ap()
    d["wqT"] = nc.dram_tensor("wqT", [DIM, DIM], F32R, kind="ExternalInput").ap()
    d["wkT"] = nc.dram_tensor("wkT", [DIM, DIM], F32R, kind="ExternalInput").ap()
    d["wvT"] = nc.dram_tensor("wvT", [DIM, DIM], BF16, kind="ExternalInput").ap()
    d["woT"] = nc.dram_tensor("woT", [DIM, DIM], BF16, kind="ExternalInput").ap()
    d["w1T"] = nc.dram_tensor("w1T", [DIM, FF], BF16, kind="ExternalInput").ap()
    d["w2T"] = nc.dram_tensor("w2T", [FF, DIM], BF16, kind="ExternalInput").ap()
    for nm, n in [("bq", DIM), ("bk", DIM), ("bo", DIM), ("b1", FF), ("b2", DIM)]:
        d[nm] = nc.dram_tensor(nm, [n], F32, kind="ExternalInput").ap()
    d["e4"] = nc.dram_tensor("e4", [4, 256], F32, kind="ExternalInput").ap()
    d["yT"] = nc.dram_tensor("yT", [DIM, NQ], F32, kind="ExternalOutput").ap()
    with tile.TileContext(nc) as tc:
        _body(nc, tc, d)
    nc.compile()
    return nc


def kernel(**inputs) -> np.ndarray:
    inputs = {k: np.asarray(v) for k, v in inputs.items()}
    x = inputs["x"].astype(np.float32)
    B, N, D = x.shape  # (2, 2048, 1024)
    w = _prep_weights(inputs)

    if "nc" not in _cache:
        _cache["nc"] = _build()
    nc = _cache["nc"]

    per_batch = N_CORES // B  # 4
    in_maps = []
    for c in range(N_CORES):
        b, chunk = divmod(c, per_batch)
        xT = np.ascontiguousarray(np.roll(x[b].T, -chunk * NQ, axis=1))
        m = {"xT": xT}
        m.update(w)
        in_maps.append(m)
    res = run_bass_kernel_spmd(nc, in_maps, core_ids=list(range(N_CORES)))
    out = np.empty((B, N, D), dtype=np.float32)
    for c in range(N_CORES):
        b, chunk = divmod(c, per_batch)
        out[b, chunk * NQ:(chunk + 1) * NQ, :] = res.results[c]["yT"].T
    return out

